# revision 23
# baseline (speedup 1.0000x reference)
"""Bass/Trainium2 kernel for nn_DeConv2d_17136919511113.

Per-(oC,iC)-pair 3-layer MLP (1->16->16->4) applied per pixel, summed over iC,
assembled into a 2x-upsampled image.  Sharding: data-parallel over batch n
(core c handles batch image c).

v5 pipeline per core (N = 64*64 = 4096 pixels, 4 superchunks of 1024):
  h1'[o,i,h,p] = max(W1*x, -b1)        DVE bf16 4x ops over 2048-px groups
                                       (b1 folded into evac bias: b2'=b2+W2@b1)
  per (o, bank) unit: 8 L2 matmuls (32x32 tiles, N=512) fill a [128,1024]
    fp32 psum tile (2 banks = two 512-px halves, same per-partition bias)
  evac: h2 = relu(z2 + b2') in ONE [128,1024] op (ACT or DVE, load-balanced)
  L3: K=64 matmuls (2 row-groups) accumulate into 2-slot l3 psum
  merge: reduce-sum over 2 slots -> yo [128,1024] (b3 added on host)
"""
import sys

sys.path.insert(0, "/opt/trn_rl_repo")

import numpy as np
import ml_dtypes

OC, IC, KH, KW, HID = 16, 16, 2, 2, 16
KK = KH * KW
N_CORES = 8
IH = IW = 64
NPX = IH * IW          # per-core pixels (one batch image)
SC = 512               # superchunk pixels
NSC = NPX // SC        # 8 superchunks
GPX = 2048             # h1 production group pixels
NG = NPX // GPX        # 2 groups
SCG = GPX // SC        # superchunks per h1 group (4)
BF16 = ml_dtypes.bfloat16

# engine cost model (ns) for static load balancing (HW-calibrated)
ACT_EVAC = 695.0
DVE_EVAC = 830.0
DVE_H1 = 680.0
DVE_MERGE = 2345.0

_CACHE = {}


def _strip_pairs(H, s):
    """h1 strip (H, s) holds pairs (i0, i0+1) with i0 = 8*H + 2*s."""
    i0 = 8 * H + 2 * s
    return i0, i0 + 1


def _build_bass():
    import concourse.bass as bass
    import concourse.mybir as mybir
    from concourse import bacc
    from concourse.tile import TileContext

    dt = mybir.dt
    Alu = mybir.AluOpType
    Act = mybir.ActivationFunctionType

    nc = bacc.Bacc(None, target_bir_lowering=False, debug=False)

    xai = nc.declare_dram_parameter("xai", [128, NPX], dt.bfloat16, isOutput=False)
    xbi = nc.declare_dram_parameter("xbi", [128, NPX], dt.bfloat16, isOutput=False)
    w1i = nc.declare_dram_parameter("w1i", [128, 32], dt.float32, isOutput=False)
    nb1i = nc.declare_dram_parameter("nb1i", [128, 32], dt.float32, isOutput=False)
    b2i = nc.declare_dram_parameter("b2i", [128, 32], dt.float32, isOutput=False)
    w2i = nc.declare_dram_parameter("w2i", [128, 1024], dt.bfloat16, isOutput=False)
    w3i = nc.declare_dram_parameter("w3i", [128, 1024], dt.bfloat16, isOutput=False)
    # device output layout [p = 32*c3 + 4*g + k, pix]; host permutes + adds b3
    yex = nc.declare_dram_parameter("y", [128, NPX], dt.float32, isOutput=True)

    # engine-balance accounting (ns)
    bal = {"act": 0.0, "dve": 0.0}

    with TileContext(nc) as tc:
        with (
            tc.tile_pool(name="singles", bufs=1) as singles,
            tc.tile_pool(name="h1p", bufs=1) as h1p,
            tc.tile_pool(name="h2p", bufs=12) as h2p,
            tc.tile_pool(name="yp", bufs=2) as yp,
            tc.tile_pool(name="pA", bufs=6, space="PSUM") as pA,
            tc.tile_pool(name="pL3", bufs=1, space="PSUM") as pL3,
        ):
            w1s = singles.tile([128, 32], dt.float32)
            nb1s = singles.tile([128, 32], dt.float32)
            b2s = singles.tile([128, 32], dt.float32)
            w2s = singles.tile([128, 1024], dt.bfloat16)
            w3s = singles.tile([128, 1024], dt.bfloat16)
            x16a = singles.tile([128, NPX], dt.bfloat16)
            x16b = singles.tile([128, NPX], dt.bfloat16)

            nc.gpsimd.dma_start(out=w1s, in_=w1i[:, :])
            nc.gpsimd.dma_start(out=nb1s, in_=nb1i[:, :])
            # x16 rows 16*il + h <- x[il] (host pre-replicated); pieces on
            # separate DMA queues so group-0 h1 starts early
            for g in range(NG):
                glo = g * GPX
                for x16, xi in ((x16a, xai), (x16b, xbi)):
                    for piece, eng in ((0, nc.sync), (1, nc.scalar)):
                        plo = glo + piece * (GPX // 2)
                        eng.dma_start(out=x16[:, plo : plo + GPX // 2], in_=xi[:, plo : plo + GPX // 2])
                if g == 0:
                    nc.gpsimd.dma_start(out=w2s, in_=w2i[:, :])
                else:
                    nc.gpsimd.dma_start(out=b2s, in_=b2i[:, :])
                    nc.gpsimd.dma_start(out=w3s, in_=w3i[:, :])

            h1T = {}
            for o in range(OC):
                for H in (0, 1):
                    h1T[(o, H)] = h1p.tile(
                        [128, GPX], dt.bfloat16,
                        tag=f"h1_{o}_{H}", name=f"h1_{o}_{H}",
                    )

            def h1_op(o, H, g, gps=False):
                # produce h1' = max(W1*x, -b1) for group g into h1T[(o,H)]
                x16 = x16a if H == 0 else x16b
                glo = g * GPX
                eng = nc.gpsimd if gps else nc.vector
                eng.tensor_scalar(
                    h1T[(o, H)],
                    x16[:, glo : glo + GPX],
                    w1s[:, 2 * o + H : 2 * o + H + 1],
                    nb1s[:, 2 * o + H : 2 * o + H + 1],
                    Alu.mult,
                    Alu.max,
                )
                if not gps:
                    bal["dve"] += DVE_H1

            h2hist = {}
            l3s = {}
            UD = 5  # L3 deferral in (o,b) units

            def emit_l2_burst(sc, o):
                # 8 MMs over 8 DISTINCT tiles: both banks' strips for this
                # 512-px superchunk (consecutive bursts never share tiles)
                gof = (sc * SC) % GPX
                pbs = {}
                for b in (0, 1):
                    pbs[b] = pA.tile([128, 512], dt.float32, tag="A", name=f"pa{o}_{b}")
                for b in (0, 1):
                    for G in (0, 1):
                        for s in (b, b + 2):
                            c = (2 * G + s // 2) ^ (o & 1)
                            nc.tensor.matmul(
                                pbs[b][32 * c : 32 * c + 32, :],
                                w2s[32 * s : 32 * s + 32, (o * 2 + G) * 32 : (o * 2 + G) * 32 + 32],
                                h1T[(o, G)][32 * s : 32 * s + 32, gof : gof + SC],
                                start=True,
                                stop=True,
                                tile_position=(32 * s, 32 * c),
                            )
                return pbs

            def emit_evacs(o, pbs, h2d):
                # one 512-wide evac per bank tile; different banks so the two
                # engines can run them concurrently
                for b in (0, 1):
                    b2col = b2s[:, 2 * o + b : 2 * o + b + 1]
                    if bal["act"] + ACT_EVAC <= bal["dve"] + DVE_EVAC:
                        nc.scalar.activation(h2d[b], pbs[b], Act.Relu, bias=b2col, scale=1.0)
                        bal["act"] += ACT_EVAC
                    else:
                        nc.vector.tensor_scalar(h2d[b], pbs[b], b2col, 0.0, Alu.add, Alu.max)
                        bal["dve"] += DVE_EVAC

            def emit_l3_pair(sc, o, b):
                # one K=64 matmul per r-slot (the two r's are concurrent)
                if sc not in l3s:
                    l3s[sc] = pL3.tile([128, 1024], dt.float32, tag="L3", name=f"l3_{sc}")
                l3 = l3s[sc]
                h2 = h2hist.pop((sc, o, b))
                grp, c3 = o // 4, o % 4
                for r in (0, 1):
                    nc.tensor.matmul(
                        l3[32 * c3 : 32 * c3 + 32, 512 * r : 512 * r + 512],
                        w3s[64 * r : 64 * r + 64, (o * 2 + b) * 32 : (o * 2 + b) * 32 + 32],
                        h2[64 * r : 64 * r + 64, :],
                        start=(grp == 0 and b == 0),
                        stop=(grp == 3 and b == 1),
                        tile_position=(64 * r, 32 * c3),
                    )

            def emit_merge(sc):
                # yo[p, j] = sum_r l3[p, 512 r + j]; b3 on host
                l3 = l3s.pop(sc)
                yo = yp.tile([128, 512], dt.float32, tag="yo")
                l3v = l3.rearrange("p (r j) -> p j r", r=2)
                nc.vector.tensor_reduce(yo, l3v, mybir.AxisListType.X, Alu.add)
                bal["dve"] += DVE_MERGE
                nc.sync.dma_start(out=yex[:, sc * SC : (sc + 1) * SC], in_=yo)

            # group-0 h1 just-in-time: only o=0,1 upfront (split into
            # 1024-px halves so the first op starts after the first DMA
            # piece); the rest are emitted two o-steps ahead in the sc0 loop
            for o in (0, 1):
                for H in (0, 1):
                    for piece in (0, 1):
                        nc.vector.tensor_scalar(
                            h1T[(o, H)][:, 1024 * piece : 1024 * piece + 1024],
                            (x16a if H == 0 else x16b)[:, 1024 * piece : 1024 * piece + 1024],
                            w1s[:, 2 * o + H : 2 * o + H + 1],
                            nb1s[:, 2 * o + H : 2 * o + H + 1],
                            Alu.mult,
                            Alu.max,
                        )
                        bal["dve"] += DVE_H1 / 2

            pairq = []      # pending L3 pairs: (sc, o, b)
            LAG = 4         # pairs kept pending (= 2 o-steps of deferral)

            def pump_pairs(n):
                for _ in range(n):
                    if len(pairq) > LAG:
                        emit_l3_pair(*pairq.pop(0))

            def flush_pairs(sc_limit):
                while pairq and pairq[0][0] < sc_limit:
                    emit_l3_pair(*pairq.pop(0))

            for sc in range(NSC):
                # finish prior superchunk's L3 + merge before any new L3
                # (pL3 bufs=1: the merge must be emitted before the next
                # superchunk's first L3 matmul re-allocates the tile)
                if sc > 0:
                    flush_pairs(sc)
                    emit_merge(sc - 1)
                for o in range(OC):
                    if sc % SCG == 0 and sc > 0:
                        k = 15 + o
                        h1_op(k // 2, k % 2, sc // SCG)
                        if o == 15:
                            h1_op(15, 1, sc // SCG)
                    h2d = {}
                    for b in (0, 1):
                        h2d[b] = h2p.tile([128, 512], dt.bfloat16, tag="h2", name=f"h2_{o}_{b}")
                        h2hist[(sc, o, b)] = h2d[b]
                    pbs = emit_l2_burst(sc, o)
                    pump_pairs(1)
                    emit_evacs(o, pbs, h2d)
                    pump_pairs(1)
                    for b in (0, 1):
                        pairq.append((sc, o, b))
                    # group-0 h1 just-in-time, two o-steps ahead
                    if sc == 0 and o + 2 < OC:
                        h1_op(o + 2, 0, 0)
                        h1_op(o + 2, 1, 0)
                    # h1 refresh for the next group, smoothed to ~1 op/step
                    # across the group's last superchunk and the next one
                    # (REF[k] legality: after o's last group-g L2, before its
                    # first group-g+1 L2; k>=15 ops are emitted in the next
                    # superchunk, handled at step top below)
                    if sc % SCG == SCG - 1 and sc + 1 < NSC and 1 <= o:
                        k = o - 1
                        if k < 15:
                            h1_op(k // 2, k % 2, sc // SCG + 1)
            flush_pairs(NSC)
            emit_merge(NSC - 1)

    nc.compile()
    return nc


def _prep_weights(W1, b1, W2, b2, W3, b3):
    """Host-side packing of weights into SBUF-image layouts (shared by all cores)."""
    w1i = np.zeros((128, 32), np.float32)
    nb1i = np.zeros((128, 32), np.float32)
    b2i = np.zeros((128, 32), np.float32)
    w2i = np.zeros((128, 1024), np.float32)
    w3i = np.zeros((128, 1024), np.float32)
    # b2' = b2 + W2 @ b1 (per (o,i,g)): compensates h1' = relu(W1 x + b1) - b1
    b2p = b2 + np.einsum("oigh,oih->oig", W2, b1)
    for o in range(OC):
        for H in (0, 1):
            # h1 group H rows: 16*il + h  -> i = 8H + il
            w1i[:, 2 * o + H] = W1[o, 8 * H : 8 * H + 8, :].reshape(128)
            nb1i[:, 2 * o + H] = -b1[o, 8 * H : 8 * H + 8, :].reshape(128)
        # L2 lhsT tiles: strip (H, s) at partitions [32s..], col block (o*2+H)
        for H in (0, 1):
            for s in range(4):
                i0, i1 = _strip_pairs(H, s)
                blk = np.zeros((32, 32), np.float32)
                blk[0:16, 0:16] = W2[o, i0].T      # lhsT[h, g] = W2[g, h]
                blk[16:32, 16:32] = W2[o, i1].T
                w2i[32 * s : 32 * s + 32, (o * 2 + H) * 32 : (o * 2 + H) * 32 + 32] = blk
        # b2 evac bias: psum block c holds strip decoded from c ^ (o&1)
        for bank in (0, 1):
            col = np.zeros(128, np.float32)
            for c in range(4):
                cc = c ^ (o & 1)
                H = cc // 2
                s = 2 * (cc % 2) + bank
                i0, i1 = _strip_pairs(H, s)
                col[32 * c : 32 * c + 16] = b2p[o, i0]
                col[32 * c + 16 : 32 * c + 32] = b2p[o, i1]
            b2i[:, 2 * o + bank] = col
        # L3 lhsT tiles (K=64): col block (2o+b); rows 64r+32ri+(ii*16+g)
        # correspond to h2 partition strip c = 2r+ri, decoded via c ^ (o&1)
        grp = o // 4
        for b in (0, 1):
            j0 = (o * 2 + b) * 32
            for r in (0, 1):
                for ri in (0, 1):
                    c = 2 * r + ri
                    cc = c ^ (o & 1)
                    H = cc // 2
                    s = 2 * (cc % 2) + b
                    i0, i1 = _strip_pairs(H, s)
                    r0 = 64 * r + 32 * ri
                    # [g, k] blocks at cols 4*grp + k
                    w3i[r0 : r0 + 16, j0 + 4 * grp : j0 + 4 * grp + 4] = W3[o, i0].T
                    w3i[r0 + 16 : r0 + 32, j0 + 4 * grp : j0 + 4 * grp + 4] = W3[o, i1].T
    b3sum = b3.sum(axis=1)  # [oC, KK]
    b3i = np.zeros((128, 1), np.float32)
    for c3 in range(4):
        for g in range(4):
            for k in range(KK):
                b3i[32 * c3 + 4 * g + k, 0] = b3sum[4 * g + c3, k]
    return {
        "w1i": w1i,
        "nb1i": nb1i,
        "b2i": b2i,
        "w2i": w2i.astype(BF16),
        "w3i": w3i.astype(BF16),
    }, b3i


def kernel(batches, W1, b1, W2, b2, W3, b3):
    from concourse.bass_utils import run_bass_kernel_spmd

    if "nc" not in _CACHE:
        _CACHE["nc"] = _build_bass()
    nc = _CACHE["nc"]

    wmaps, b3i = _prep_weights(
        np.asarray(W1, np.float32), np.asarray(b1, np.float32),
        np.asarray(W2, np.float32), np.asarray(b2, np.float32),
        np.asarray(W3, np.float32), np.asarray(b3, np.float32),
    )
    batches = np.asarray(batches, np.float32)
    n = batches.shape[0]
    assert n == N_CORES
    in_maps = []
    for cidx in range(N_CORES):
        xr = batches[cidx].reshape(IC, 1, NPX).astype(BF16)
        xr = np.broadcast_to(xr, (IC, HID, NPX)).reshape(2, 128, NPX)
        in_maps.append({"xai": np.ascontiguousarray(xr[0]), "xbi": np.ascontiguousarray(xr[1]), **wmaps})
    res = run_bass_kernel_spmd(nc, in_maps, list(range(N_CORES)))
    out = np.empty((N_CORES, OC, KH * IH, KW * IW), np.float32)
    for cidx in range(N_CORES):
        ydev = res.results[cidx]["y"].astype(np.float32) + b3i
        # partition p = 32*c3 + 4*g + k (k = 2*kh + kw, o = 4*g + c3);
        # rows 16..31 of each 32-block are padding
        yd = ydev.reshape(4, 32, IH, IW)[:, :16]
        yd = yd.reshape(4, 4, KH, KW, IH, IW)          # [c3, g, kh, kw, ih, iw]
        out[cidx] = yd.transpose(1, 0, 4, 2, 5, 3).reshape(OC, KH * IH, KW * IW)
    return out
